# revision 10
# baseline (speedup 1.0000x reference)
"""Causal attention (B=4, N=2048, D=1024) on 8 Trainium2 NeuronCores.

Sharding: core 2b+p handles batch b; the two cores of a batch split the KEY
tiles by parity (core p owns key tiles {p, p+2, ..., p+14}).  Each core
projects Q for all 16 query tiles but K/V only for its 8 owned key tiles,
computes unnormalized partial attention (exp-weights @ V) plus per-row
exp-sums, and the host merges:  out = (O_0 + O_1) / (s_0 + s_1).
This halves the K/V projection work vs. batch-only sharding.

The program is SPMD-uniform: the host permutes x tiles per core (owned
tiles first, in causal order, then the rest), so program slot s < 8 is the
core's s-th owned query tile (attends its first s+1 owned key tiles, with a
triangular mask on the last = diagonal) and slot s >= 8 is the (s-8)-th
other-parity query tile (attends s-7 owned tiles; for the odd-parity core
the last of those is a pad, masked to -inf via per-core mask data).

Everything runs in bfloat16 on the PE (full rate, cheap transposes), with
f32 PSUM accumulation and f32 outputs.  All tensors (Q^T, K^T, V, weights)
stay SBUF-resident; x is loaded once; there are no DRAM spills.
"""
import sys

sys.path.insert(0, "/opt/trn_rl_repo")

from contextlib import ExitStack

import ml_dtypes
import numpy as np

import concourse.bass as bass
import concourse.mybir as mybir
import concourse.tile as tile
from concourse import bacc
from concourse.bass_utils import run_bass_kernel_spmd
from concourse.masks import make_identity

B, N, D = 4, 2048, 1024
N_CORES = 8
N_TILES = 16         # 128-token tiles per batch
SCALE = 1.0 / 32.0   # 1/sqrt(D)
NEG = -1.0e9

F32 = mybir.dt.float32
BF16 = mybir.dt.bfloat16
BF = ml_dtypes.bfloat16

_NC_CACHE = {}
TRACE = False
LAST_EXEC_NS = None


def _build_nc():
    nc = bacc.Bacc(None, target_bir_lowering=False, debug=False)

    # All inputs partition-major so DMA descriptors are large contiguous runs.
    # x pre-transposed + per-core tile-permuted on host:
    # [partition(d%128), slot, dchunk, token]
    xt = nc.declare_dram_parameter("xt", [128, N_TILES, 8, 128], BF16, isOutput=False)
    # wq/wk: [p(d%128), echunk, dchunk, ecol]; wv: [p, dchunk, ehalf, ecol]
    wq = nc.declare_dram_parameter("wq", [128, 8, 8, 128], BF16, isOutput=False)
    wk = nc.declare_dram_parameter("wk", [128, 8, 8, 128], BF16, isOutput=False)
    wv = nc.declare_dram_parameter("wv", [128, 8, 2, 512], BF16, isOutput=False)
    # masks[:,0]: causal tri (shared); masks[:,1]: zeros (even core) / -1e9 (odd)
    masks = nc.declare_dram_parameter("masks", [128, 2, 128], F32, isOutput=False)
    out_o = nc.declare_dram_parameter("out_o", [N_TILES, 128, D], F32, isOutput=True)
    out_s = nc.declare_dram_parameter("out_s", [128, N_TILES], F32, isOutput=True)

    with tile.TileContext(nc) as tc, ExitStack() as top:
        consts = top.enter_context(tc.tile_pool(name="consts", bufs=1))
        res = top.enter_context(tc.tile_pool(name="res", bufs=1))
        xt_pool = top.enter_context(tc.tile_pool(name="xtp", bufs=1))
        p_pool = top.enter_context(tc.tile_pool(name="pp", bufs=2))
        pt_pool = top.enter_context(tc.tile_pool(name="ptp", bufs=2))
        out_pool = top.enter_context(tc.tile_pool(name="op", bufs=2))
        ps = top.enter_context(tc.tile_pool(name="ps", bufs=1, space="PSUM"))

        ident_f = consts.tile([128, 128], F32)
        make_identity(nc, ident_f)
        ident = consts.tile([128, 128], BF16)
        nc.vector.tensor_copy(ident, ident_f)
        mask_sb = consts.tile([128, 2, 128], F32)
        nc.sync.dma_start(out=mask_sb, in_=masks[:, :, :])

        # SBUF residents
        QT = res.tile([128, 16, 8, 128], BF16)   # [e%128, slot, echunk, token]
        KT = res.tile([128, 8, 1024], BF16)      # [e%128, echunk, key(j*128+kk)]
        V = res.tile([128, 8, 1024], BF16)       # [token%128, tile j, e]
        wq_sb = res.tile([128, 8, 8, 128], BF16)  # [d%128, echunk, dchunk, ecol]
        wk_sb = res.tile([128, 8, 8, 128], BF16)
        wv_sb = res.tile([128, 8, 2, 512], BF16)  # [d%128, dchunk, ehalf, ecol]
        rsums = res.tile([128, 16], F32)

        # weight DMAs (scalar HWDGE queue, in order of first use); x batches on
        # the sync HWDGE queue so the first Q matmul's inputs load in parallel
        nc.scalar.dma_start(out=wq_sb[:, 0:1, :, :], in_=wq[:, 0:1, :, :])

        def load_x(bi):
            xT = xt_pool.tile([128, 4, 8, 128], BF16, tag=f"xT{bi}", name=f"x{bi}")
            nc.sync.dma_start(out=xT, in_=xt[:, bi * 4:bi * 4 + 4, :, :])
            return xT

        xbufs = [load_x(bi) for bi in range(4)]
        nc.scalar.dma_start(out=wq_sb[:, 1:8, :, :], in_=wq[:, 1:8, :, :])
        nc.scalar.dma_start(out=wk_sb, in_=wk[:, :, :, :])
        nc.scalar.dma_start(out=wv_sb, in_=wv[:, :, :, :])

        def proj_q(bi, xT):
            """Q^T for program slots bi*4 .. bi*4+3."""
            for e in range(8):
                qps = ps.tile([128, 512], F32, tag="acc", bufs=2, name=f"q{bi}_{e}")
                for c in range(8):
                    nc.tensor.matmul(
                        qps, wq_sb[:, e, c, :], xT[:, :, c, :],
                        start=(c == 0), stop=(c == 7),
                    )
                nc.vector.tensor_copy(
                    QT[:, bi * 4:bi * 4 + 4, e, :],
                    qps.rearrange("p (j q) -> p j q", j=4),
                )

        def proj_kv(bi, xT):
            """K^T and V for owned tiles bi*4 .. bi*4+3 (bi in {0,1})."""
            for e in range(8):
                kps = ps.tile([128, 512], F32, tag="acc", bufs=2, name=f"k{bi}_{e}")
                for c in range(8):
                    nc.tensor.matmul(
                        kps, wk_sb[:, e, c, :], xT[:, :, c, :],
                        start=(c == 0), stop=(c == 7),
                    )
                nc.vector.tensor_copy(KT[:, e, bi * 512:bi * 512 + 512], kps)
            for j in range(4):
                for eh in range(2):
                    vps = ps.tile([128, 512], F32, tag="acc", bufs=2,
                                  name=f"v{bi}_{j}_{eh}")
                    for c in range(8):
                        nc.tensor.matmul(
                            vps, xT[:, j, c, :], wv_sb[:, c, eh, :],
                            start=(c == 0), stop=(c == 7),
                        )
                    nc.vector.tensor_copy(
                        V[:, bi * 4 + j, eh * 512:eh * 512 + 512], vps
                    )

        def emit_av(prev):
            s, L, P_sb = prev
            O_ps = ps.tile([128, D], F32, tag="O", bufs=1, name=f"O{s}")
            for kt in range(L):
                ptps = ps.tile([128, 128], BF16, tag="acc", bufs=2, name=f"tp{s}_{kt}")
                nc.tensor.transpose(ptps, P_sb[:, kt * 128:(kt + 1) * 128], ident)
                pt_sb = pt_pool.tile([128, 128], BF16, tag="pt", name=f"pt{s}_{kt}")
                nc.vector.tensor_copy(pt_sb, ptps)
                for h in range(2):
                    nc.tensor.matmul(
                        O_ps[:, h * 512:(h + 1) * 512], pt_sb,
                        V[:, kt, h * 512:(h + 1) * 512],
                        start=(kt == 0), stop=(kt == L - 1),
                    )
            out_sb = out_pool.tile([128, D], F32, tag="osb", name=f"ou{s}")
            nc.vector.tensor_copy(out_sb, O_ps)
            eng = nc.sync if s % 2 == 0 else nc.scalar
            eng.dma_start(out=out_o[s][:, :], in_=out_sb)

        def do_slot(s, prev):
            L = (s % 8) + 1
            S_ps = ps.tile([128, L * 128], F32, tag="S", bufs=2, name=f"S{s}")
            for kg in range((L * 128 + 511) // 512):
                w = min(512, L * 128 - kg * 512)
                for e in range(8):
                    nc.tensor.matmul(
                        S_ps[:, kg * 512:kg * 512 + w],
                        QT[:, s, e, :],
                        KT[:, e, kg * 512:kg * 512 + w],
                        start=(e == 0), stop=(e == 7),
                    )
            mi = 0 if s < 8 else 1
            nc.vector.tensor_add(
                S_ps[:, (L - 1) * 128:L * 128],
                S_ps[:, (L - 1) * 128:L * 128],
                mask_sb[:, mi, :],
            )
            # |scores|/32 is small; exp without max-subtraction, fused row-sum
            P_sb = p_pool.tile([128, L * 128], BF16, tag="P", name=f"P{s}")
            nc.scalar.activation(
                P_sb, S_ps, mybir.ActivationFunctionType.Exp,
                bias=0.0, scale=SCALE, accum_out=rsums[:, s:s + 1],
            )
            if prev is not None:
                emit_av(prev)
            return (s, L, P_sb)

        # ---- schedule ----
        proj_q(0, xbufs[0])
        proj_kv(0, xbufs[0])
        proj_q(1, xbufs[1])
        proj_kv(1, xbufs[1])

        # slot order within each attention block hides exp latency under the
        # previous slot's (bigger) AV; phase B ends on small slots so the
        # final AV + out-DMA tail is short.
        prev = None
        for s in (3, 0, 2, 1):      # owned slots 0..3 (need only O1/O2)
            prev = do_slot(s, prev)
        proj_q(2, xbufs[2])         # slots 8..11
        for s in (7, 4, 6, 5):
            prev = do_slot(s, prev)
        proj_q(3, xbufs[3])         # slots 12..15
        for s in (15, 14, 13, 12, 11, 8, 10, 9):
            prev = do_slot(s, prev)
        emit_av(prev)
        nc.sync.dma_start(out=out_s[:, :], in_=rsums)

    nc.compile()
    return nc


def _tri_mask():
    q = np.arange(128)[:, None]
    k = np.arange(128)[None, :]
    return np.where(k <= q, 0.0, NEG).astype(np.float32)


def kernel(x, Wq, Wk, Wv):
    global LAST_EXEC_NS
    x = np.ascontiguousarray(np.asarray(x, dtype=np.float32))
    Wq = np.ascontiguousarray(np.asarray(Wq, dtype=np.float32))
    Wk = np.ascontiguousarray(np.asarray(Wk, dtype=np.float32))
    Wv = np.ascontiguousarray(np.asarray(Wv, dtype=np.float32))

    if "nc" not in _NC_CACHE:
        _NC_CACHE["nc"] = _build_nc()
    nc = _NC_CACHE["nc"]

    # host pre-transpose: x[b] (N, D) -> (p=d%128, tile, dchunk, token), bf16,
    # partition-major so each DMA descriptor covers a long contiguous run
    xt_all = np.ascontiguousarray(
        x.reshape(B, N_TILES, 128, 8, 128).transpose(0, 4, 1, 3, 2).astype(BF)
    )  # [B, p, tile, c, q]
    wq_r = np.ascontiguousarray(Wq.reshape(8, 128, 8, 128).transpose(1, 2, 0, 3).astype(BF))
    wk_r = np.ascontiguousarray(Wk.reshape(8, 128, 8, 128).transpose(1, 2, 0, 3).astype(BF))
    wv_r = np.ascontiguousarray(Wv.reshape(8, 128, 2, 512).transpose(1, 0, 2, 3).astype(BF))

    tri = _tri_mask()
    zero = np.zeros((128, 128), np.float32)
    neg = np.full((128, 128), NEG, np.float32)
    in_maps = []
    for c in range(N_CORES):
        b, p = divmod(c, 2)
        perm = list(range(p, 16, 2)) + list(range(1 - p, 16, 2))
        in_maps.append({
            "xt": np.ascontiguousarray(xt_all[b][:, perm]),
            "wq": wq_r, "wk": wk_r, "wv": wv_r,
            "masks": np.ascontiguousarray(
                np.stack([tri, zero if p == 0 else neg], axis=1)),
        })

    res = run_bass_kernel_spmd(nc, in_maps, list(range(N_CORES)), trace=TRACE)
    LAST_EXEC_NS = res.exec_time_ns

    # host softmax-merge: out = (O_even + O_odd) / (s_even + s_odd)
    Osum = np.zeros((B, N_TILES, 128, D), np.float32)
    Ssum = np.zeros((B, N_TILES, 128), np.float32)
    for c in range(N_CORES):
        b, p = divmod(c, 2)
        oo = res.results[c]["out_o"]
        ss = res.results[c]["out_s"]
        for s in range(N_TILES):
            q = 2 * (s % 8) + (p if s < 8 else 1 - p)
            Osum[b, q] += oo[s]
            Ssum[b, q] += ss[:, s]
    out = Osum / Ssum[..., None]
    return np.ascontiguousarray(out.reshape(B, N, D))
